# revision 1
# baseline (speedup 1.0000x reference)
"""Trainium2 Bass kernel for nn_CausalConvolution (dense_cnn).

Reference computation (B=4, S=4096, H=2048, CIN=COUT=4096, K=4, G=8):
    h   = x @ W_in.T + b_in                       # [B,S,CIN]
    y   = silu(causal_grouped_conv1d(h) + conv_b) # [B,S,COUT], groups=8, k=4
    out = y @ W_out.T + b_out                     # [B,S,H]

Sharding: one conv group per NeuronCore (G = 8 = n_cores).
Core g computes channels [g*512, (g+1)*512) of h (column-parallel W_in),
its conv group (512 in / 512 out channels), and a row-parallel partial of
the output projection. Host sums the 8 partials and adds b_out. No
cross-core communication on device.

All matmuls run in bf16 (fp32 PSUM accumulation); everything is kept in
"transposed" [channel, time] layout on-chip so the contraction dim always
sits on SBUF partitions without any on-chip transposes.

Schedule notes: PE is the bottleneck (6144 N=512 matmuls/core ~= 1.31 ms
at the bf16 streaming limit), so the kernel front-loads only the DMAs the
first matmuls need (w_in + first x tile), gates the conv/out weights
behind them, pre-warms the PE clock (HAM) with scratch matmuls during the
initial DMA wait, and runs stage 1 one time-tile ahead of stages 2/3.
"""

import numpy as np
import ml_dtypes

# Problem constants (hardcoded per the harness contract).
B, S, H = 4, 4096, 2048
CIN = COUT = 4096
KT = 4          # conv taps
G = 8           # conv groups == number of cores
CG = CIN // G   # 512 channels per group/core
T = B * S       # 16384 flattened time steps
NCORES = 8

HK = H // 128       # 16 contraction chunks for stage 1
CT = CG // 128      # 4 chunks of the per-core channel dim
TTILE = 512         # time-tile (N of every matmul)
NH = H // TTILE     # 4 output-column chunks of stage 3

_BF16 = ml_dtypes.bfloat16

_CACHE = {}

# test.py introspection: the most recent BassKernelResults from a run.
LAST_RESULTS = None


def _build_nc():
    import concourse.bass as bass
    import concourse.mybir as mybir
    import concourse.tile as tile
    from concourse.tile import add_dep_helper
    from concourse import bacc

    dt = mybir.dt
    AF = mybir.ActivationFunctionType

    nc = bacc.Bacc(
        "TRN2", target_bir_lowering=False, debug=False, num_devices=NCORES
    )

    xT = nc.dram_tensor("xT", [128, HK, T], dt.bfloat16, kind="ExternalInput")
    w_in = nc.dram_tensor("w_in", [128, CT, HK, 128], dt.bfloat16, kind="ExternalInput")
    cw = nc.dram_tensor("cw", [128, KT, CT, CG], dt.bfloat16, kind="ExternalInput")
    wo = nc.dram_tensor("wo", [128, CT, H], dt.bfloat16, kind="ExternalInput")
    b_in = nc.dram_tensor("b_in", [128, CT], dt.float32, kind="ExternalInput")
    cb = nc.dram_tensor("cb", [128, CT], dt.float32, kind="ExternalInput")
    out = nc.dram_tensor("out", [T, H], dt.float32, kind="ExternalOutput")

    n_tt = S // TTILE  # time tiles per batch

    with tile.TileContext(nc) as tc:
        # PE warmup: dep-free matmuls on scratch data run while the first
        # weight/x DMAs are in flight, so HAM un-throttles (K=8/8) before
        # the real matmul stream begins.
        with (
            tc.tile_pool(name="warm", bufs=1) as warmpool,
            tc.tile_pool(name="warmps", bufs=1, space="PSUM") as warmpspool,
        ):
            scratch = warmpool.tile([128, 640], dt.bfloat16)
            nc.vector.memset(scratch[:], 0.0)
            wps = warmpspool.tile([128, TTILE], dt.float32)
            for _ in range(22):
                nc.tensor.matmul(
                    wps[:], scratch[:, 0:128], scratch[:, 128:640],
                    start=True, stop=True,
                )
        with (
            tc.tile_pool(name="weights", bufs=1) as wpool,
            tc.tile_pool(name="xin", bufs=3) as xpool,
            tc.tile_pool(name="hbuf", bufs=2) as hpool,
            tc.tile_pool(name="ybuf", bufs=3) as ypool,
            tc.tile_pool(name="obuf", bufs=2) as opool,
            tc.tile_pool(name="ps1", bufs=2, space="PSUM") as ps1pool,
            tc.tile_pool(name="ps2", bufs=2, space="PSUM") as ps2pool,
            tc.tile_pool(name="ps3", bufs=4, space="PSUM") as ps3pool,
        ):
            # Startup DMA scheduling. Two facts drive the shape: (1) a
            # single dma_start descriptor streams on one DMA engine at
            # only ~65 GB/s, so anything urgent must be SPLIT into
            # several descriptors for parallel engine pickup; (2) all
            # in-flight descriptors share HBM bandwidth fairly, so bulk
            # loads must be GATED behind the urgent ones or everything
            # finishes late together.
            # Phase A0 (ungated): stage-1 tile-0's c=0 weights + full
            # first x tile, as 12 parallel descriptors. Later phases are
            # ordered by consumption deadline: c=1 lands ~1.5 µs after
            # A0, c=2/3 after that — each just ahead of stage 1's use.
            w_in_sb = wpool.tile([128, CT, HK, 128], dt.bfloat16)
            xt_first = xpool.tile([128, HK, TTILE], dt.bfloat16, tag="xt")
            for half in range(2):
                a0d = nc.sync.dma_start(
                    w_in_sb[:, 0, 8 * half : 8 * half + 8, :],
                    w_in[:, 0, 8 * half : 8 * half + 8, :],
                )
            for q in range(8):
                a0d = nc.sync.dma_start(
                    xt_first[:, 2 * q : 2 * q + 2, :],
                    xT[:, 2 * q : 2 * q + 2, 0:TTILE],
                )
            # Phase A1a (gated on A0): c=1 weights only.
            for half in range(2):
                a1ad = nc.sync.dma_start(
                    w_in_sb[:, 1, 8 * half : 8 * half + 8, :],
                    w_in[:, 1, 8 * half : 8 * half + 8, :],
                )
                add_dep_helper(a1ad.ins, a0d.ins, reason="phase A1a")
            # Phase A1b (gated on A1a): the rest of stage-1's weights.
            bin_sb = wpool.tile([128, CT], dt.float32)
            bd = nc.sync.dma_start(bin_sb[:], b_in[:])
            add_dep_helper(bd.ins, a1ad.ins, reason="phase A1b")
            for cc in range(2, CT):
                for half in range(2):
                    a1d = nc.sync.dma_start(
                        w_in_sb[:, cc, 8 * half : 8 * half + 8, :],
                        w_in[:, cc, 8 * half : 8 * half + 8, :],
                    )
                    add_dep_helper(a1d.ins, a1ad.ins, reason="phase A1b")
            cb_sb = wpool.tile([128, CT], dt.float32)
            cbd = nc.sync.dma_start(cb_sb[:], cb[:])
            add_dep_helper(cbd.ins, a0d.ins, reason="phase A1")
            # Bulk weights, deferred further (needed only after one /
            # two full stage-1 tiles respectively), 4 descriptors each.
            cw_sb = wpool.tile([128, KT, CT, CG], dt.bfloat16)
            for k in range(KT):
                cwd = nc.sync.dma_start(cw_sb[:, k], cw[:, k])
                add_dep_helper(cwd.ins, a1d.ins, reason="defer conv weights")
            wo_sb = wpool.tile([128, CT, H], dt.bfloat16)
            for oo in range(CT):
                wod = nc.sync.dma_start(wo_sb[:, oo], wo[:, oo])
                add_dep_helper(wod.ins, cwd.ins, reason="defer out weights")

            tiles = [(b, tt) for b in range(B) for tt in range(n_tt)]
            hts = {}   # batch -> hT tile
            yts = {}   # (b, tt) -> y tile

            def stage1(b, tt):
                t0 = tt * TTILE
                tg = b * S + t0
                if tt == 0:
                    # h^T for this batch: [c, t] with a 3-column zero halo
                    # in front so causal taps at batch start read zeros.
                    hts[b] = hpool.tile(
                        [128, CT, KT - 1 + S], dt.bfloat16, tag="hT", name="hT"
                    )
                    nc.vector.memset(hts[b][:, :, 0 : KT - 1], 0.0)
                hT = hts[b]
                if b == 0 and tt == 0:
                    xt = xt_first
                else:
                    xt = xpool.tile([128, HK, TTILE], dt.bfloat16, tag="xt")
                    nc.sync.dma_start(xt[:, 0:8, :], xT[:, 0:8, tg : tg + TTILE])
                    nc.sync.dma_start(xt[:, 8:16, :], xT[:, 8:16, tg : tg + TTILE])
                for c in range(CT):
                    ps = ps1pool.tile([128, TTILE], dt.float32)
                    for hk in range(HK):
                        nc.tensor.matmul(
                            ps[:],
                            w_in_sb[:, c, hk, :],
                            xt[:, hk, :],
                            start=(hk == 0),
                            stop=(hk == HK - 1),
                        )
                    nc.scalar.activation(
                        hT[:, c, KT - 1 + t0 : KT - 1 + t0 + TTILE],
                        ps[:],
                        AF.Identity,
                        bias=bin_sb[:, c : c + 1],
                    )

            def stage23(b, tt):
                t0 = tt * TTILE
                tg = b * S + t0
                hT = hts[b]
                # Stage 2: causal grouped conv as 16 accumulated matmuls
                yt = ypool.tile([128, CT, TTILE], dt.bfloat16, tag="yt")
                for o in range(CT):
                    ps = ps2pool.tile([128, TTILE], dt.float32)
                    n_acc = KT * CT
                    acc = 0
                    for ik in range(CT):
                        for k in range(KT):
                            nc.tensor.matmul(
                                ps[:],
                                cw_sb[:, k, ik, o * 128 : (o + 1) * 128],
                                hT[:, ik, t0 + k : t0 + k + TTILE],
                                start=(acc == 0),
                                stop=(acc == n_acc - 1),
                            )
                            acc += 1
                    nc.scalar.activation(
                        yt[:, o, :],
                        ps[:],
                        AF.Silu,
                        bias=cb_sb[:, o : o + 1],
                    )
                # Stage 3: partial out[t, :] = y^T.T @ W_out_g^T
                last_tile = b == B - 1 and tt == n_tt - 1
                for ss in range(TTILE // 128):
                    ot = opool.tile([128, H], dt.float32, tag="ot")
                    for nh in range(NH):
                        ps = ps3pool.tile([128, TTILE], dt.float32)
                        for oo in range(CT):
                            nc.tensor.matmul(
                                ps[:],
                                yt[:, oo, ss * 128 : (ss + 1) * 128],
                                wo_sb[:, oo, nh * TTILE : (nh + 1) * TTILE],
                                start=(oo == 0),
                                stop=(oo == CT - 1),
                            )
                        nc.vector.tensor_copy(
                            ot[:, nh * TTILE : (nh + 1) * TTILE], ps[:]
                        )
                        row = tg + ss * 128
                        # The final tile's stores sit on the kernel-exit
                        # critical path (one descriptor ~= one DMA engine
                        # at ~65 GB/s): split them for parallel drain.
                        n_split = 4 if last_tile else 1
                        w = TTILE // n_split
                        for sp in range(n_split):
                            col = nh * TTILE + sp * w
                            nc.sync.dma_start(
                                out[row : row + 128, col : col + w],
                                ot[:, col : col + w],
                            )

            # Stage 1 runs one time-tile ahead of stages 2/3: keeps the PE
            # stream dense and moves the cw/wo DMA deadlines out by a tile.
            for i, (b, tt) in enumerate(tiles):
                stage1(b, tt)
                if i > 0:
                    stage23(*tiles[i - 1])
            stage23(*tiles[-1])

    nc.compile()
    return nc


def _prep_inputs(x, W_in, b_in, conv_w, conv_b, W_out):
    """Host-side shard + transpose + bf16 cast. Returns in_maps for 8 cores."""
    x = np.asarray(x, dtype=np.float32)
    # x^T in [h_inner=128, h_outer, t] layout
    xr = (
        x.reshape(T, HK, 128).transpose(2, 1, 0).astype(_BF16)
    )  # [128, HK, T]
    xr = np.ascontiguousarray(xr)

    in_maps = []
    for g in range(NCORES):
        c0 = g * CG
        w_in_g = np.ascontiguousarray(
            np.asarray(W_in[c0 : c0 + CG, :])
            .reshape(CT, 128, HK, 128)
            .transpose(3, 0, 2, 1)
            .astype(_BF16)
        )  # [128, CT, HK, 128]: (hi, cc, hk, ci) = W_in[c0+cc*128+ci, hk*128+hi]
        cw_g = np.ascontiguousarray(
            np.asarray(conv_w[c0 : c0 + CG, :, :])
            .reshape(CG, CT, 128, KT)
            .transpose(2, 3, 1, 0)
            .astype(_BF16)
        )  # [128, KT, CT, CG]: (ii, k, io, o) = conv_w[c0+o, io*128+ii, k]
        wo_g = np.ascontiguousarray(
            np.asarray(W_out[:, c0 : c0 + CG])
            .reshape(H, CT, 128)
            .transpose(2, 1, 0)
            .astype(_BF16)
        )  # [128, CT, H]: (oi, oo, h) = W_out[h, c0+oo*128+oi]
        bin_g = np.ascontiguousarray(
            np.asarray(b_in[c0 : c0 + CG], dtype=np.float32).reshape(CT, 128).T
        )  # [128, CT]
        cb_g = np.ascontiguousarray(
            np.asarray(conv_b[c0 : c0 + CG], dtype=np.float32).reshape(CT, 128).T
        )
        in_maps.append(
            {
                "xT": xr,
                "w_in": w_in_g,
                "cw": cw_g,
                "wo": wo_g,
                "b_in": bin_g,
                "cb": cb_g,
            }
        )
    return in_maps


def kernel(x, W_in, b_in, conv_w, conv_b, W_out, b_out):
    global LAST_RESULTS
    from concourse import bass_utils

    if "nc" not in _CACHE:
        _CACHE["nc"] = _build_nc()
    nc = _CACHE["nc"]

    in_maps = _prep_inputs(x, W_in, b_in, conv_w, conv_b, W_out)

    res = bass_utils.run_bass_kernel_spmd(
        nc, in_maps, core_ids=list(range(NCORES))
    )
    LAST_RESULTS = res

    acc = np.array(res.results[0]["out"], dtype=np.float32, copy=True)
    for r in res.results[1:]:
        acc += r["out"]
    acc += np.asarray(b_out, dtype=np.float32)[None, :]
    return acc.reshape(B, S, H)



# revision 7
# speedup vs baseline: 1.0134x; 1.0134x over previous
"""Trainium2 Bass kernel for nn_CausalConvolution (dense_cnn).

Reference computation (B=4, S=4096, H=2048, CIN=COUT=4096, K=4, G=8):
    h   = x @ W_in.T + b_in                       # [B,S,CIN]
    y   = silu(causal_grouped_conv1d(h) + conv_b) # [B,S,COUT], groups=8, k=4
    out = y @ W_out.T + b_out                     # [B,S,H]

Sharding: one conv group per NeuronCore (G = 8 = n_cores).
Core g computes channels [g*512, (g+1)*512) of h (column-parallel W_in),
its conv group (512 in / 512 out channels), and a row-parallel partial of
the output projection. Host sums the 8 partials and adds b_out. No
cross-core communication on device.

All matmuls run in bf16 (fp32 PSUM accumulation); everything is kept in
"transposed" [channel, time] layout on-chip so the contraction dim always
sits on SBUF partitions without any on-chip transposes.

Schedule notes: PE is the bottleneck (6144 N=512 matmuls/core ~= 1.31 ms
at the bf16 streaming limit), so the kernel front-loads only the DMAs the
first matmuls need (w_in + first x tile), gates the conv/out weights
behind them, pre-warms the PE clock (HAM) with scratch matmuls during the
initial DMA wait, and runs stage 1 one time-tile ahead of stages 2/3.
"""

import numpy as np
import ml_dtypes

# Problem constants (hardcoded per the harness contract).
B, S, H = 4, 4096, 2048
CIN = COUT = 4096
KT = 4          # conv taps
G = 8           # conv groups == number of cores
CG = CIN // G   # 512 channels per group/core
T = B * S       # 16384 flattened time steps
NCORES = 8

HK = H // 128       # 16 contraction chunks for stage 1
CT = CG // 128      # 4 chunks of the per-core channel dim
TTILE = 512         # time-tile (N of every matmul)
NH = H // TTILE     # 4 output-column chunks of stage 3

_BF16 = ml_dtypes.bfloat16

_CACHE = {}

# test.py introspection: the most recent BassKernelResults from a run.
LAST_RESULTS = None


def _build_nc():
    import concourse.bass as bass
    import concourse.mybir as mybir
    import concourse.tile as tile
    from concourse.tile import add_dep_helper
    from concourse import bacc

    dt = mybir.dt
    AF = mybir.ActivationFunctionType

    nc = bacc.Bacc(
        "TRN2", target_bir_lowering=False, debug=False, num_devices=NCORES
    )

    xT = nc.dram_tensor("xT", [128, HK, T], dt.bfloat16, kind="ExternalInput")
    w_in = nc.dram_tensor("w_in", [128, CT, HK, 128], dt.bfloat16, kind="ExternalInput")
    cw = nc.dram_tensor("cw", [128, KT, CT, CG], dt.bfloat16, kind="ExternalInput")
    wo = nc.dram_tensor("wo", [128, CT, H], dt.bfloat16, kind="ExternalInput")
    b_in = nc.dram_tensor("b_in", [128, CT], dt.float32, kind="ExternalInput")
    cb = nc.dram_tensor("cb", [128, CT], dt.float32, kind="ExternalInput")
    # Per-core partials are summed on the host in fp32; storing them in
    # bf16 halves the store traffic and costs ~2e-4 extra rel err.
    out = nc.dram_tensor("out", [T, H], dt.bfloat16, kind="ExternalOutput")

    n_tt = S // TTILE  # time tiles per batch

    with tile.TileContext(nc) as tc:
        # PE warmup: dep-free matmuls on scratch data run while the first
        # weight/x DMAs are in flight, so HAM un-throttles (K=8/8) before
        # the real matmul stream begins.
        with (
            tc.tile_pool(name="warm", bufs=1) as warmpool,
            tc.tile_pool(name="warmps", bufs=1, space="PSUM") as warmpspool,
        ):
            scratch = warmpool.tile([128, 640], dt.bfloat16)
            nc.vector.memset(scratch[:], 0.0)
            wps = warmpspool.tile([128, TTILE], dt.float32)
            for _ in range(38):
                nc.tensor.matmul(
                    wps[:], scratch[:, 0:128], scratch[:, 128:640],
                    start=True, stop=True,
                )
        with (
            tc.tile_pool(name="weights", bufs=1) as wpool,
            tc.tile_pool(name="xin", bufs=3) as xpool,
            tc.tile_pool(name="hbuf", bufs=2) as hpool,
            tc.tile_pool(name="ybuf", bufs=3) as ypool,
            tc.tile_pool(name="obuf", bufs=4) as opool,
            tc.tile_pool(name="ps1", bufs=2, space="PSUM") as ps1pool,
            tc.tile_pool(name="ps2", bufs=2, space="PSUM") as ps2pool,
            tc.tile_pool(name="ps3", bufs=4, space="PSUM") as ps3pool,
        ):
            # Startup DMA scheduling. Three facts drive the shape: (1) a
            # single dma_start descriptor streams on one DMA engine at
            # only ~65 GB/s, so anything urgent must be SPLIT into
            # several descriptors for parallel engine pickup; (2) the
            # issuing queue spends ~0.65 µs per descriptor, so urgent
            # descriptors are also split across BOTH hwdge queues
            # (sync + scalar) and issued before everything else; (3)
            # in-flight descriptors share HBM bandwidth fairly, so bulk
            # loads are GATED behind the urgent ones.
            # Phase A0: stage-1 tile-0's c=0 weights (sync) + the first
            # x tile (hk 0-7 on sync, 8-15 on scalar), lowest-hk first
            # so the first matmuls' inputs land first.
            w_in_sb = wpool.tile([128, CT, HK, 128], dt.bfloat16)
            xt_first = xpool.tile([128, HK, TTILE], dt.bfloat16, tag="xt")
            a0_sync = []
            a0d = nc.sync.dma_start(w_in_sb[:, 0, 0:8, :], w_in[:, 0, 0:8, :])
            a0_sync.append(a0d)
            for q in range(4):
                a0d = nc.sync.dma_start(
                    xt_first[:, 2 * q : 2 * q + 2, :],
                    xT[:, 2 * q : 2 * q + 2, 0:TTILE],
                )
                a0_sync.append(a0d)
            a0s = nc.scalar.dma_start(w_in_sb[:, 0, 8:16, :], w_in[:, 0, 8:16, :])
            for q in range(4, 8):
                a0s = nc.scalar.dma_start(
                    xt_first[:, 2 * q : 2 * q + 2, :],
                    xT[:, 2 * q : 2 * q + 2, 0:TTILE],
                )
            # Phase A1a (gated on A0): c=1 weights only.
            a1a = []
            for half, eng in ((0, nc.sync), (1, nc.scalar)):
                a1ad = eng.dma_start(
                    w_in_sb[:, 1, 8 * half : 8 * half + 8, :],
                    w_in[:, 1, 8 * half : 8 * half + 8, :],
                )
                add_dep_helper(a1ad.ins, a0d.ins, reason="phase A1a")
                add_dep_helper(a1ad.ins, a0s.ins, reason="phase A1a")
                a1a.append(a1ad)
            # Phase A1b (gated on A1a): the rest of stage-1's weights.
            bin_sb = wpool.tile([128, CT], dt.float32)
            bd = nc.sync.dma_start(bin_sb[:], b_in[:])
            add_dep_helper(bd.ins, a1a[0].ins, reason="phase A1b")
            for cc in range(2, CT):
                for half, eng in ((0, nc.sync), (1, nc.scalar)):
                    a1d = eng.dma_start(
                        w_in_sb[:, cc, 8 * half : 8 * half + 8, :],
                        w_in[:, cc, 8 * half : 8 * half + 8, :],
                    )
                    add_dep_helper(a1d.ins, a1a[half].ins, reason="phase A1b")
            cb_sb = wpool.tile([128, CT], dt.float32)
            cbd = nc.scalar.dma_start(cb_sb[:], cb[:])
            add_dep_helper(cbd.ins, a1a[1].ins, reason="phase A1")
            # Bulk weights, deferred further (needed only after one /
            # two full stage-1 tiles respectively), 4 descriptors each.
            cw_sb = wpool.tile([128, KT, CT, CG], dt.bfloat16)
            for k in range(KT):
                eng = nc.sync if k % 2 == 0 else nc.scalar
                cwd = eng.dma_start(cw_sb[:, k], cw[:, k])
                add_dep_helper(cwd.ins, a1d.ins, reason="defer conv weights")
            wo_sb = wpool.tile([128, CT, H], dt.bfloat16)
            for oo in range(CT):
                eng = nc.sync if oo % 2 == 0 else nc.scalar
                wod = eng.dma_start(wo_sb[:, oo], wo[:, oo])
                add_dep_helper(wod.ins, cwd.ins, reason="defer out weights")

            tiles = [(b, tt) for b in range(B) for tt in range(n_tt)]
            hts = {}   # batch -> hT tile
            yts = {}   # (b, tt) -> y tile

            def stage1(b, tt):
                t0 = tt * TTILE
                tg = b * S + t0
                if tt == 0:
                    # h^T for this batch: [c, t] with a 3-column zero halo
                    # in front so causal taps at batch start read zeros.
                    hts[b] = hpool.tile(
                        [128, CT, KT - 1 + S], dt.bfloat16, tag="hT", name="hT"
                    )
                    nc.vector.memset(hts[b][:, :, 0 : KT - 1], 0.0)
                hT = hts[b]
                if b == 0 and tt == 0:
                    xt = xt_first
                else:
                    xt = xpool.tile([128, HK, TTILE], dt.bfloat16, tag="xt")
                    nc.sync.dma_start(xt[:, 0:8, :], xT[:, 0:8, tg : tg + TTILE])
                    nc.sync.dma_start(xt[:, 8:16, :], xT[:, 8:16, tg : tg + TTILE])
                for c in range(CT):
                    ps = ps1pool.tile([128, TTILE], dt.float32)
                    for hk in range(HK):
                        nc.tensor.matmul(
                            ps[:],
                            w_in_sb[:, c, hk, :],
                            xt[:, hk, :],
                            start=(hk == 0),
                            stop=(hk == HK - 1),
                        )
                    nc.scalar.activation(
                        hT[:, c, KT - 1 + t0 : KT - 1 + t0 + TTILE],
                        ps[:],
                        AF.Identity,
                        bias=bin_sb[:, c : c + 1],
                    )

            def stage23(b, tt):
                t0 = tt * TTILE
                tg = b * S + t0
                hT = hts[b]
                # Stage 2: causal grouped conv as 16 accumulated matmuls
                yt = ypool.tile([128, CT, TTILE], dt.bfloat16, tag="yt")
                for o in range(CT):
                    ps = ps2pool.tile([128, TTILE], dt.float32)
                    n_acc = KT * CT
                    acc = 0
                    for ik in range(CT):
                        for k in range(KT):
                            nc.tensor.matmul(
                                ps[:],
                                cw_sb[:, k, ik, o * 128 : (o + 1) * 128],
                                hT[:, ik, t0 + k : t0 + k + TTILE],
                                start=(acc == 0),
                                stop=(acc == n_acc - 1),
                            )
                            acc += 1
                    nc.scalar.activation(
                        yt[:, o, :],
                        ps[:],
                        AF.Silu,
                        bias=cb_sb[:, o : o + 1],
                    )
                # Stage 3: partial out[t, :] = y^T.T @ W_out_g^T
                for ss in range(TTILE // 128):
                    ot = opool.tile([128, H], dt.bfloat16, tag="ot")
                    for nh in range(NH):
                        ps = ps3pool.tile([128, TTILE], dt.float32)
                        for oo in range(CT):
                            nc.tensor.matmul(
                                ps[:],
                                yt[:, oo, ss * 128 : (ss + 1) * 128],
                                wo_sb[:, oo, nh * TTILE : (nh + 1) * TTILE],
                                start=(oo == 0),
                                stop=(oo == CT - 1),
                            )
                        nc.vector.tensor_copy(
                            ot[:, nh * TTILE : (nh + 1) * TTILE], ps[:]
                        )
                    row = tg + ss * 128
                    # Two half-row stores, one per hwdge queue: halves the
                    # per-queue issue serialization and lets two DMA
                    # engines drain each block in parallel.
                    nc.sync.dma_start(
                        out[row : row + 128, 0 : H // 2], ot[:, 0 : H // 2]
                    )
                    nc.scalar.dma_start(
                        out[row : row + 128, H // 2 : H], ot[:, H // 2 : H]
                    )

            # Stage 1 runs one time-tile ahead of stages 2/3: keeps the PE
            # stream dense and moves the cw/wo DMA deadlines out by a tile.
            for i, (b, tt) in enumerate(tiles):
                stage1(b, tt)
                if i > 0:
                    stage23(*tiles[i - 1])
            stage23(*tiles[-1])

    nc.compile()
    return nc


def _prep_inputs(x, W_in, b_in, conv_w, conv_b, W_out):
    """Host-side shard + transpose + bf16 cast. Returns in_maps for 8 cores."""
    x = np.asarray(x, dtype=np.float32)
    # x^T in [h_inner=128, h_outer, t] layout
    xr = (
        x.reshape(T, HK, 128).transpose(2, 1, 0).astype(_BF16)
    )  # [128, HK, T]
    xr = np.ascontiguousarray(xr)

    in_maps = []
    for g in range(NCORES):
        c0 = g * CG
        w_in_g = np.ascontiguousarray(
            np.asarray(W_in[c0 : c0 + CG, :])
            .reshape(CT, 128, HK, 128)
            .transpose(3, 0, 2, 1)
            .astype(_BF16)
        )  # [128, CT, HK, 128]: (hi, cc, hk, ci) = W_in[c0+cc*128+ci, hk*128+hi]
        cw_g = np.ascontiguousarray(
            np.asarray(conv_w[c0 : c0 + CG, :, :])
            .reshape(CG, CT, 128, KT)
            .transpose(2, 3, 1, 0)
            .astype(_BF16)
        )  # [128, KT, CT, CG]: (ii, k, io, o) = conv_w[c0+o, io*128+ii, k]
        wo_g = np.ascontiguousarray(
            np.asarray(W_out[:, c0 : c0 + CG])
            .reshape(H, CT, 128)
            .transpose(2, 1, 0)
            .astype(_BF16)
        )  # [128, CT, H]: (oi, oo, h) = W_out[h, c0+oo*128+oi]
        bin_g = np.ascontiguousarray(
            np.asarray(b_in[c0 : c0 + CG], dtype=np.float32).reshape(CT, 128).T
        )  # [128, CT]
        cb_g = np.ascontiguousarray(
            np.asarray(conv_b[c0 : c0 + CG], dtype=np.float32).reshape(CT, 128).T
        )
        in_maps.append(
            {
                "xT": xr,
                "w_in": w_in_g,
                "cw": cw_g,
                "wo": wo_g,
                "b_in": bin_g,
                "cb": cb_g,
            }
        )
    return in_maps


def kernel(x, W_in, b_in, conv_w, conv_b, W_out, b_out):
    global LAST_RESULTS
    from concourse import bass_utils

    if "nc" not in _CACHE:
        _CACHE["nc"] = _build_nc()
    nc = _CACHE["nc"]

    in_maps = _prep_inputs(x, W_in, b_in, conv_w, conv_b, W_out)

    res = bass_utils.run_bass_kernel_spmd(
        nc, in_maps, core_ids=list(range(NCORES))
    )
    LAST_RESULTS = res

    acc = np.asarray(res.results[0]["out"]).astype(np.float32)
    for r in res.results[1:]:
        acc += np.asarray(r["out"]).astype(np.float32)
    acc += np.asarray(b_out, dtype=np.float32)[None, :]
    return acc.reshape(B, S, H)



# revision 11
# speedup vs baseline: 1.0590x; 1.0450x over previous
"""Trainium2 Bass kernel for nn_CausalConvolution (dense_cnn).

Reference computation (B=4, S=4096, H=2048, CIN=COUT=4096, K=4, G=8):
    h   = x @ W_in.T + b_in                       # [B,S,CIN]
    y   = silu(causal_grouped_conv1d(h) + conv_b) # [B,S,COUT], groups=8, k=4
    out = y @ W_out.T + b_out                     # [B,S,H]

Sharding: one conv group per NeuronCore (G = 8 = n_cores).
Core g computes channels [g*512, (g+1)*512) of h (column-parallel W_in),
its conv group (512 in / 512 out channels), and a row-parallel partial of
the output projection. Host sums the 8 partials (stored bf16) and adds
b_out. No cross-core communication on device.

All matmuls run with the contraction dim on SBUF partitions in a
"transposed" [channel, time] layout, bf16 with fp32 PSUM accumulation —
except the last N8 (=4) of stage 1's 16 contraction chunks, which run as
fp8e4 DoubleRow pairs (2 chunks per matmul, 2x PE throughput). To let
fp8 and bf16 products share one PSUM accumulation, ALL stage-1 operands
are pre-scaled by powers of two on the host (x*8, W_in*256; exact in
bf16), and the stage-1 activation applies 1/2048. rel_err budget: e4m3
on 4/16 chunks costs ~1.7e-2 of the 2e-2 allowance (measured in sim).

Schedule notes (from perfetto analysis of prior revisions):
- PE is the bottleneck: 5632 bf16 N=512 matmuls + 256 fp8 DoubleRow
  matmuls/core ~= 1.26 ms streaming floor. Everything else hides behind
  it or dies trying.
- DMA descriptors drain roughly FIFO per hwdge ring with bandwidth
  shared across all in-flight descriptors, so ISSUE ORDER is the
  scheduling tool: the sync ring carries x tiles (+ half the output
  stores), the scalar ring carries all weights in deadline order
  (w_in c0, c1, bias, c2, c3, conv, out) + the other half of stores.
- The PE warmup scratch must come from the persistent weight pool: a
  scratch in its own pool gets its SBUF reused for w_in, and the WAR
  dependency then blocks the critical first weight DMA until warmup
  ends (cost ~10 us, found the hard way).
- Stage 1 runs 3 tiles ahead of stage 2, stage 3 one tile behind
  stage 2, so the conv/out weights' arrival deadlines (~30/~45 us)
  clear while the PE chews through x-only work at the start.
"""

import numpy as np
import ml_dtypes

# Problem constants (hardcoded per the harness contract).
B, S, H = 4, 4096, 2048
CIN = COUT = 4096
KT = 4          # conv taps
G = 8           # conv groups == number of cores
CG = CIN // G   # 512 channels per group/core
T = B * S       # 16384 flattened time steps
NCORES = 8

HK = H // 128       # 16 contraction chunks for stage 1
N8 = 4              # stage-1 chunks done in fp8 DoubleRow (must be even)
HK16 = HK - N8      # stage-1 chunks done in bf16
CT = CG // 128      # 4 chunks of the per-core channel dim
TTILE = 512         # time-tile (N of every matmul)
NH = H // TTILE     # 4 output-column chunks of stage 3

SCALE_X = 8.0       # stage-1 operand pre-scales (powers of two, exact in bf16)
SCALE_W = 256.0
SCALE_INV = 1.0 / (SCALE_X * SCALE_W)

D1 = 3              # stage2 runs D1 tiles behind stage1
D2 = 1              # stage3 runs D2 tiles behind stage2

_BF16 = ml_dtypes.bfloat16
_F8 = ml_dtypes.float8_e4m3  # IEEE e4m3, max +-240 == TRN FP8_EXP4

_CACHE = {}

# test.py introspection: the most recent BassKernelResults from a run.
LAST_RESULTS = None


def _build_nc():
    import concourse.bass as bass
    import concourse.mybir as mybir
    import concourse.tile as tile
    from concourse import bacc

    dt = mybir.dt
    AF = mybir.ActivationFunctionType
    DR = mybir.MatmulPerfMode.DoubleRow

    nc = bacc.Bacc(
        "TRN2", target_bir_lowering=False, debug=False, num_devices=NCORES
    )

    xT16 = nc.dram_tensor("xT16", [128, HK16, T], dt.bfloat16, kind="ExternalInput")
    xT8 = nc.dram_tensor("xT8", [128, N8, T], dt.float8e4, kind="ExternalInput")
    w_in16 = nc.dram_tensor(
        "w_in16", [128, CT, HK16, 128], dt.bfloat16, kind="ExternalInput"
    )
    w_in8 = nc.dram_tensor(
        "w_in8", [128, CT, N8, 128], dt.float8e4, kind="ExternalInput"
    )
    cw = nc.dram_tensor("cw", [128, KT, CT, CG], dt.bfloat16, kind="ExternalInput")
    wo = nc.dram_tensor("wo", [128, CT, H], dt.bfloat16, kind="ExternalInput")
    b_in = nc.dram_tensor("b_in", [128, CT], dt.float32, kind="ExternalInput")
    cb = nc.dram_tensor("cb", [128, CT], dt.float32, kind="ExternalInput")
    # Per-core partials are summed on the host in fp32; storing them in
    # bf16 halves the store traffic and costs ~2e-4 extra rel err.
    out = nc.dram_tensor("out", [T, H], dt.bfloat16, kind="ExternalOutput")

    n_tt = S // TTILE  # time tiles per batch

    with tile.TileContext(nc) as tc:
        with (
            tc.tile_pool(name="weights", bufs=1) as wpool,
            tc.tile_pool(name="xin", bufs=4) as xpool,
            tc.tile_pool(name="x8in", bufs=4) as x8pool,
            tc.tile_pool(name="hbuf", bufs=2) as hpool,
            tc.tile_pool(name="ybuf", bufs=3) as ypool,
            tc.tile_pool(name="obuf", bufs=4) as opool,
            tc.tile_pool(name="ps1", bufs=2, space="PSUM") as ps1pool,
            tc.tile_pool(name="ps2", bufs=2, space="PSUM") as ps2pool,
            tc.tile_pool(name="ps3", bufs=4, space="PSUM") as ps3pool,
        ):
            # PE warmup: dep-free matmuls on scratch run while the first
            # weight/x DMAs are in flight, so HAM un-throttles (K=8/8)
            # before the real matmul stream begins. The scratch lives in
            # the persistent pool — see module docstring.
            scratch = wpool.tile([128, 640], dt.bfloat16)
            nc.vector.memset(scratch[:], 0.0)
            wps = ps3pool.tile([128, TTILE], dt.float32, tag="ps3")
            for _ in range(24):
                nc.tensor.matmul(
                    wps[:], scratch[:, 0:128], scratch[:, 128:640],
                    start=True, stop=True,
                )

            # ---- startup DMAs: ring order == drain order == priority ----
            # sync ring: the first x tile, lowest chunks first.
            xt_first = xpool.tile([128, HK16, TTILE], dt.bfloat16, tag="xt")
            x8_first = x8pool.tile([128, N8, TTILE], dt.float8e4, tag="xt8")
            for q in range(4):
                nc.sync.dma_start(
                    xt_first[:, 3 * q : 3 * q + 3, :],
                    xT16[:, 3 * q : 3 * q + 3, 0:TTILE],
                )
            nc.sync.dma_start(x8_first[:], xT8[:, :, 0:TTILE])
            # scalar ring: all weights, in consumption-deadline order.
            w16_sb = wpool.tile([128, CT, HK16, 128], dt.bfloat16)
            w8_sb = wpool.tile([128, CT, N8, 128], dt.float8e4)
            bin_sb = wpool.tile([128, CT], dt.float32)
            cb_sb = wpool.tile([128, CT], dt.float32)
            cw_sb = wpool.tile([128, KT, CT, CG], dt.bfloat16)
            wo_sb = wpool.tile([128, CT, H], dt.bfloat16)
            for half in range(2):
                nc.scalar.dma_start(
                    w16_sb[:, 0, 6 * half : 6 * half + 6, :],
                    w_in16[:, 0, 6 * half : 6 * half + 6, :],
                )
            nc.scalar.dma_start(w8_sb[:], w_in8[:])
            nc.scalar.dma_start(w16_sb[:, 1], w_in16[:, 1])
            nc.scalar.dma_start(bin_sb[:], b_in[:])
            nc.scalar.dma_start(cb_sb[:], cb[:])
            nc.scalar.dma_start(w16_sb[:, 2], w_in16[:, 2])
            nc.scalar.dma_start(w16_sb[:, 3], w_in16[:, 3])
            for k in range(KT):
                nc.scalar.dma_start(cw_sb[:, k], cw[:, k])
            for oo in range(CT):
                nc.scalar.dma_start(wo_sb[:, oo], wo[:, oo])

            tiles = [(b, tt) for b in range(B) for tt in range(n_tt)]
            hts = {}   # batch -> hT tile
            yts = {}   # (b, tt) -> y tile

            def stage1(b, tt):
                t0 = tt * TTILE
                tg = b * S + t0
                if tt == 0:
                    # h^T for this batch: [c, t] with a 3-column zero halo
                    # in front so causal taps at batch start read zeros.
                    hts[b] = hpool.tile(
                        [128, CT, KT - 1 + S], dt.bfloat16, tag="hT", name="hT"
                    )
                    nc.vector.memset(hts[b][:, :, 0 : KT - 1], 0.0)
                hT = hts[b]
                if b == 0 and tt == 0:
                    xt, x8t = xt_first, x8_first
                else:
                    xt = xpool.tile([128, HK16, TTILE], dt.bfloat16, tag="xt")
                    x8t = x8pool.tile([128, N8, TTILE], dt.float8e4, tag="xt8")
                    nc.sync.dma_start(xt[:, 0:6, :], xT16[:, 0:6, tg : tg + TTILE])
                    nc.sync.dma_start(xt[:, 6:12, :], xT16[:, 6:12, tg : tg + TTILE])
                    nc.sync.dma_start(x8t[:], xT8[:, :, tg : tg + TTILE])
                for c in range(CT):
                    ps = ps1pool.tile([128, TTILE], dt.float32)
                    for hk in range(HK16):
                        nc.tensor.matmul(
                            ps[:],
                            w16_sb[:, c, hk, :],
                            xt[:, hk, :],
                            start=(hk == 0),
                            stop=False,
                        )
                    for j in range(N8 // 2):
                        nc.tensor.matmul(
                            ps[:],
                            w8_sb[:, c, 2 * j : 2 * j + 2, :],
                            x8t[:, 2 * j : 2 * j + 2, :],
                            start=False,
                            stop=(j == N8 // 2 - 1),
                            perf_mode=DR,
                        )
                    nc.scalar.activation(
                        hT[:, c, KT - 1 + t0 : KT - 1 + t0 + TTILE],
                        ps[:],
                        AF.Identity,
                        bias=bin_sb[:, c : c + 1],
                        scale=SCALE_INV,
                    )

            def stage2(b, tt):
                t0 = tt * TTILE
                hT = hts[b]
                # causal grouped conv as 16 accumulated matmuls per chunk
                yt = ypool.tile([128, CT, TTILE], dt.bfloat16, tag="yt")
                yts[(b, tt)] = yt
                for o in range(CT):
                    ps = ps2pool.tile([128, TTILE], dt.float32)
                    n_acc = KT * CT
                    acc = 0
                    for ik in range(CT):
                        for k in range(KT):
                            nc.tensor.matmul(
                                ps[:],
                                cw_sb[:, k, ik, o * 128 : (o + 1) * 128],
                                hT[:, ik, t0 + k : t0 + k + TTILE],
                                start=(acc == 0),
                                stop=(acc == n_acc - 1),
                            )
                            acc += 1
                    nc.scalar.activation(
                        yt[:, o, :],
                        ps[:],
                        AF.Silu,
                        bias=cb_sb[:, o : o + 1],
                    )

            def stage3(b, tt):
                tg = b * S + tt * TTILE
                yt = yts.pop((b, tt))
                for ss in range(TTILE // 128):
                    ot = opool.tile([128, H], dt.bfloat16, tag="ot")
                    for nh in range(NH):
                        ps = ps3pool.tile([128, TTILE], dt.float32, tag="ps3")
                        for oo in range(CT):
                            nc.tensor.matmul(
                                ps[:],
                                yt[:, oo, ss * 128 : (ss + 1) * 128],
                                wo_sb[:, oo, nh * TTILE : (nh + 1) * TTILE],
                                start=(oo == 0),
                                stop=(oo == CT - 1),
                            )
                        nc.vector.tensor_copy(
                            ot[:, nh * TTILE : (nh + 1) * TTILE], ps[:]
                        )
                    row = tg + ss * 128
                    # Two half-row stores, one per hwdge ring.
                    nc.sync.dma_start(
                        out[row : row + 128, 0 : H // 2], ot[:, 0 : H // 2]
                    )
                    nc.scalar.dma_start(
                        out[row : row + 128, H // 2 : H], ot[:, H // 2 : H]
                    )

            n = len(tiles)
            for i in range(n):
                stage1(*tiles[i])
                if i >= D1:
                    stage2(*tiles[i - D1])
                if i >= D1 + D2:
                    stage3(*tiles[i - D1 - D2])
            for i in range(n - D1, n):
                stage2(*tiles[i])
                stage3(*tiles[i - D2])
            for i in range(n - D2, n):
                stage3(*tiles[i])

    nc.compile()
    return nc


def _prep_inputs(x, W_in, b_in, conv_w, conv_b, W_out):
    """Host-side shard + transpose + cast. Returns in_maps for 8 cores."""
    x = np.asarray(x, dtype=np.float32)
    xr = x.reshape(T, HK, 128).transpose(2, 1, 0)  # [128, HK, T]
    xr16 = np.ascontiguousarray(xr[:, :HK16, :] * SCALE_X).astype(_BF16)
    xr8 = np.ascontiguousarray(
        np.clip(xr[:, HK16:, :] * SCALE_X, -240, 240)
    ).astype(_F8)

    in_maps = []
    for g in range(NCORES):
        c0 = g * CG
        w_in_g = (
            np.asarray(W_in[c0 : c0 + CG, :])
            .reshape(CT, 128, HK, 128)
            .transpose(3, 0, 2, 1)
        ) * SCALE_W  # [128, CT, HK, 128]: (hi, cc, hk, ci)
        w16_g = np.ascontiguousarray(w_in_g[:, :, :HK16, :]).astype(_BF16)
        w8_g = np.ascontiguousarray(
            np.clip(w_in_g[:, :, HK16:, :], -240, 240)
        ).astype(_F8)
        cw_g = np.ascontiguousarray(
            np.asarray(conv_w[c0 : c0 + CG, :, :])
            .reshape(CG, CT, 128, KT)
            .transpose(2, 3, 1, 0)
            .astype(_BF16)
        )  # [128, KT, CT, CG]: (ii, k, io, o) = conv_w[c0+o, io*128+ii, k]
        wo_g = np.ascontiguousarray(
            np.asarray(W_out[:, c0 : c0 + CG])
            .reshape(H, CT, 128)
            .transpose(2, 1, 0)
            .astype(_BF16)
        )  # [128, CT, H]: (oi, oo, h) = W_out[h, c0+oo*128+oi]
        bin_g = np.ascontiguousarray(
            np.asarray(b_in[c0 : c0 + CG], dtype=np.float32).reshape(CT, 128).T
        )  # [128, CT]
        cb_g = np.ascontiguousarray(
            np.asarray(conv_b[c0 : c0 + CG], dtype=np.float32).reshape(CT, 128).T
        )
        in_maps.append(
            {
                "xT16": xr16,
                "xT8": xr8,
                "w_in16": w16_g,
                "w_in8": w8_g,
                "cw": cw_g,
                "wo": wo_g,
                "b_in": bin_g,
                "cb": cb_g,
            }
        )
    return in_maps


def kernel(x, W_in, b_in, conv_w, conv_b, W_out, b_out):
    global LAST_RESULTS
    from concourse import bass_utils

    if "nc" not in _CACHE:
        _CACHE["nc"] = _build_nc()
    nc = _CACHE["nc"]

    in_maps = _prep_inputs(x, W_in, b_in, conv_w, conv_b, W_out)

    res = bass_utils.run_bass_kernel_spmd(
        nc, in_maps, core_ids=list(range(NCORES))
    )
    LAST_RESULTS = res

    acc = np.asarray(res.results[0]["out"]).astype(np.float32)
    for r in res.results[1:]:
        acc += np.asarray(r["out"]).astype(np.float32)
    acc += np.asarray(b_out, dtype=np.float32)[None, :]
    return acc.reshape(B, S, H)


# revision 13
# speedup vs baseline: 1.0694x; 1.0099x over previous
"""Trainium2 Bass kernel for nn_CausalConvolution (dense_cnn).

Reference computation (B=4, S=4096, H=2048, CIN=COUT=4096, K=4, G=8):
    h   = x @ W_in.T + b_in                       # [B,S,CIN]
    y   = silu(causal_grouped_conv1d(h) + conv_b) # [B,S,COUT], groups=8, k=4
    out = y @ W_out.T + b_out                     # [B,S,H]

Sharding: one conv group per NeuronCore (G = 8 = n_cores).
Core g computes channels [g*512, (g+1)*512) of h (column-parallel W_in),
its conv group (512 in / 512 out channels), and a row-parallel partial of
the output projection. Host sums the 8 partials (stored bf16) and adds
b_out. No cross-core communication on device.

All matmuls run with the contraction dim on SBUF partitions in a
"transposed" [channel, time] layout, bf16 with fp32 PSUM accumulation —
except the last N8 (=4) of stage 1's 16 contraction chunks, which run as
fp8e4 DoubleRow pairs (2 chunks per matmul, 2x PE throughput). To let
fp8 and bf16 products share one PSUM accumulation, ALL stage-1 operands
are pre-scaled by powers of two on the host (x*8, W_in*256; exact in
bf16), and the stage-1 activation applies 1/2048. rel_err budget: e4m3
on 4/16 chunks costs ~1.7e-2 of the 2e-2 allowance (measured in sim).

Schedule notes (from perfetto analysis of prior revisions):
- PE is the bottleneck: 5632 bf16 N=512 matmuls + 256 fp8 DoubleRow
  matmuls/core ~= 1.26 ms streaming floor. Everything else hides behind
  it or dies trying.
- DMA descriptors drain roughly FIFO per hwdge ring with bandwidth
  shared across all in-flight descriptors, so ISSUE ORDER is the
  scheduling tool: the sync ring carries x tiles (+ half the output
  stores), the scalar ring carries all weights in deadline order
  (w_in c0, c1, bias, c2, c3, conv, out) + the other half of stores.
- The PE warmup scratch must come from the persistent weight pool: a
  scratch in its own pool gets its SBUF reused for w_in, and the WAR
  dependency then blocks the critical first weight DMA until warmup
  ends (cost ~10 us, found the hard way).
- Stage 1 runs 3 tiles ahead of stage 2, stage 3 one tile behind
  stage 2, so the conv/out weights' arrival deadlines (~30/~45 us)
  clear while the PE chews through x-only work at the start.
"""

import numpy as np
import ml_dtypes

# Problem constants (hardcoded per the harness contract).
B, S, H = 4, 4096, 2048
CIN = COUT = 4096
KT = 4          # conv taps
G = 8           # conv groups == number of cores
CG = CIN // G   # 512 channels per group/core
T = B * S       # 16384 flattened time steps
NCORES = 8

HK = H // 128       # 16 contraction chunks for stage 1
N8 = 4              # stage-1 chunks done in fp8 DoubleRow (must be even)
HK16 = HK - N8      # stage-1 chunks done in bf16
CT = CG // 128      # 4 chunks of the per-core channel dim
TTILE = 512         # time-tile (N of every matmul)
NH = H // TTILE     # 4 output-column chunks of stage 3

SCALE_X = 8.0       # stage-1 operand pre-scales (powers of two, exact in bf16)
SCALE_W = 256.0
SCALE_INV = 1.0 / (SCALE_X * SCALE_W)

D1 = 3              # stage2 runs D1 tiles behind stage1
D2 = 1              # stage3 runs D2 tiles behind stage2

_BF16 = ml_dtypes.bfloat16
_F8 = ml_dtypes.float8_e4m3  # IEEE e4m3, max +-240 == TRN FP8_EXP4

_CACHE = {}

# test.py introspection: the most recent BassKernelResults from a run.
LAST_RESULTS = None


def _build_nc():
    import concourse.bass as bass
    import concourse.mybir as mybir
    import concourse.tile as tile
    from concourse import bacc

    dt = mybir.dt
    AF = mybir.ActivationFunctionType
    DR = mybir.MatmulPerfMode.DoubleRow

    nc = bacc.Bacc(
        "TRN2", target_bir_lowering=False, debug=False, num_devices=NCORES
    )

    xT16 = nc.dram_tensor("xT16", [128, HK16, T], dt.bfloat16, kind="ExternalInput")
    xT8 = nc.dram_tensor("xT8", [128, N8, T], dt.float8e4, kind="ExternalInput")
    w_in16 = nc.dram_tensor(
        "w_in16", [128, CT, HK16, 128], dt.bfloat16, kind="ExternalInput"
    )
    w_in8 = nc.dram_tensor(
        "w_in8", [128, CT, N8, 128], dt.float8e4, kind="ExternalInput"
    )
    cw = nc.dram_tensor("cw", [128, KT, CT, CG], dt.bfloat16, kind="ExternalInput")
    wo = nc.dram_tensor("wo", [128, CT, H], dt.bfloat16, kind="ExternalInput")
    b_in = nc.dram_tensor("b_in", [128, CT], dt.float32, kind="ExternalInput")
    cb = nc.dram_tensor("cb", [128, CT], dt.float32, kind="ExternalInput")
    # Per-core partials are summed on the host in fp32; storing them in
    # bf16 halves the store traffic and costs ~2e-4 extra rel err.
    out = nc.dram_tensor("out", [T, H], dt.bfloat16, kind="ExternalOutput")

    n_tt = S // TTILE  # time tiles per batch

    with tile.TileContext(nc) as tc:
        with (
            tc.tile_pool(name="weights", bufs=1) as wpool,
            tc.tile_pool(name="xin", bufs=4) as xpool,
            tc.tile_pool(name="x8in", bufs=4) as x8pool,
            tc.tile_pool(name="hbuf", bufs=2) as hpool,
            tc.tile_pool(name="ybuf", bufs=3) as ypool,
            tc.tile_pool(name="obuf", bufs=4) as opool,
            tc.tile_pool(name="ps1", bufs=2, space="PSUM") as ps1pool,
            tc.tile_pool(name="ps2", bufs=2, space="PSUM") as ps2pool,
            tc.tile_pool(name="ps3", bufs=4, space="PSUM") as ps3pool,
        ):
            # PE warmup: dep-free matmuls on scratch run while the first
            # weight/x DMAs are in flight, so HAM un-throttles (K=8/8)
            # before the real matmul stream begins. The scratch lives in
            # the persistent pool — see module docstring.
            scratch = wpool.tile([128, 640], dt.bfloat16)
            nc.vector.memset(scratch[:], 0.0)
            wps = ps3pool.tile([128, TTILE], dt.float32, tag="ps3")
            for _ in range(10):
                nc.tensor.matmul(
                    wps[:], scratch[:, 0:128], scratch[:, 128:640],
                    start=True, stop=True,
                )

            # ---- startup DMAs: ring order == drain order == priority ----
            # The startup is HBM-bandwidth-bound (stage 1 runs D1 tiles
            # ahead, consuming x at ~4x the steady rate), so descriptors
            # are ordered by consumption deadline. Weights go on the sync
            # ring; the first x tile on the scalar ring, which must be
            # drained of DMA issues before the first ACTIVATE needs it
            # (DMA backpressure on the issuing queue blocks later queue
            # entries). cw/wo are emitted mid-loop (after stage1(3)) so
            # they ride behind xt(1..3) in the sync ring.
            xt_first = xpool.tile([128, HK16, TTILE], dt.bfloat16, tag="xt")
            x8_first = x8pool.tile([128, N8, TTILE], dt.float8e4, tag="xt8")
            for q in range(4):
                nc.scalar.dma_start(
                    xt_first[:, 3 * q : 3 * q + 3, :],
                    xT16[:, 3 * q : 3 * q + 3, 0:TTILE],
                )
            nc.scalar.dma_start(x8_first[:], xT8[:, :, 0:TTILE])
            w16_sb = wpool.tile([128, CT, HK16, 128], dt.bfloat16)
            w8_sb = wpool.tile([128, CT, N8, 128], dt.float8e4)
            bin_sb = wpool.tile([128, CT], dt.float32)
            cb_sb = wpool.tile([128, CT], dt.float32)
            cw_sb = wpool.tile([128, KT, CT, CG], dt.bfloat16)
            wo_sb = wpool.tile([128, CT, H], dt.bfloat16)
            for half in range(2):
                nc.sync.dma_start(
                    w16_sb[:, 0, 6 * half : 6 * half + 6, :],
                    w_in16[:, 0, 6 * half : 6 * half + 6, :],
                )
            nc.sync.dma_start(w8_sb[:], w_in8[:])
            nc.sync.dma_start(w16_sb[:, 1], w_in16[:, 1])
            nc.sync.dma_start(bin_sb[:], b_in[:])
            nc.sync.dma_start(cb_sb[:], cb[:])
            nc.sync.dma_start(w16_sb[:, 2], w_in16[:, 2])
            nc.sync.dma_start(w16_sb[:, 3], w_in16[:, 3])

            tiles = [(b, tt) for b in range(B) for tt in range(n_tt)]
            hts = {}   # batch -> hT tile
            yts = {}   # (b, tt) -> y tile

            def stage1(b, tt):
                t0 = tt * TTILE
                tg = b * S + t0
                if tt == 0:
                    # h^T for this batch: [c, t] with a 3-column zero halo
                    # in front so causal taps at batch start read zeros.
                    hts[b] = hpool.tile(
                        [128, CT, KT - 1 + S], dt.bfloat16, tag="hT", name="hT"
                    )
                    nc.vector.memset(hts[b][:, :, 0 : KT - 1], 0.0)
                hT = hts[b]
                if b == 0 and tt == 0:
                    xt, x8t = xt_first, x8_first
                else:
                    xt = xpool.tile([128, HK16, TTILE], dt.bfloat16, tag="xt")
                    x8t = x8pool.tile([128, N8, TTILE], dt.float8e4, tag="xt8")
                    nc.sync.dma_start(xt[:, 0:6, :], xT16[:, 0:6, tg : tg + TTILE])
                    nc.sync.dma_start(xt[:, 6:12, :], xT16[:, 6:12, tg : tg + TTILE])
                    nc.sync.dma_start(x8t[:], xT8[:, :, tg : tg + TTILE])
                for c in range(CT):
                    ps = ps1pool.tile([128, TTILE], dt.float32)
                    for hk in range(HK16):
                        nc.tensor.matmul(
                            ps[:],
                            w16_sb[:, c, hk, :],
                            xt[:, hk, :],
                            start=(hk == 0),
                            stop=False,
                        )
                    for j in range(N8 // 2):
                        nc.tensor.matmul(
                            ps[:],
                            w8_sb[:, c, 2 * j : 2 * j + 2, :],
                            x8t[:, 2 * j : 2 * j + 2, :],
                            start=False,
                            stop=(j == N8 // 2 - 1),
                            perf_mode=DR,
                        )
                    nc.scalar.activation(
                        hT[:, c, KT - 1 + t0 : KT - 1 + t0 + TTILE],
                        ps[:],
                        AF.Identity,
                        bias=bin_sb[:, c : c + 1],
                        scale=SCALE_INV,
                    )

            def stage2(b, tt):
                t0 = tt * TTILE
                hT = hts[b]
                # causal grouped conv as 16 accumulated matmuls per chunk
                yt = ypool.tile([128, CT, TTILE], dt.bfloat16, tag="yt")
                yts[(b, tt)] = yt
                for o in range(CT):
                    ps = ps2pool.tile([128, TTILE], dt.float32)
                    n_acc = KT * CT
                    acc = 0
                    for ik in range(CT):
                        for k in range(KT):
                            nc.tensor.matmul(
                                ps[:],
                                cw_sb[:, k, ik, o * 128 : (o + 1) * 128],
                                hT[:, ik, t0 + k : t0 + k + TTILE],
                                start=(acc == 0),
                                stop=(acc == n_acc - 1),
                            )
                            acc += 1
                    nc.scalar.activation(
                        yt[:, o, :],
                        ps[:],
                        AF.Silu,
                        bias=cb_sb[:, o : o + 1],
                    )

            def stage3(b, tt):
                tg = b * S + tt * TTILE
                yt = yts.pop((b, tt))
                for ss in range(TTILE // 128):
                    ot = opool.tile([128, H], dt.bfloat16, tag="ot")
                    for nh in range(NH):
                        ps = ps3pool.tile([128, TTILE], dt.float32, tag="ps3")
                        for oo in range(CT):
                            nc.tensor.matmul(
                                ps[:],
                                yt[:, oo, ss * 128 : (ss + 1) * 128],
                                wo_sb[:, oo, nh * TTILE : (nh + 1) * TTILE],
                                start=(oo == 0),
                                stop=(oo == CT - 1),
                            )
                        nc.vector.tensor_copy(
                            ot[:, nh * TTILE : (nh + 1) * TTILE], ps[:]
                        )
                    row = tg + ss * 128
                    # Two half-row stores, one per hwdge ring.
                    nc.sync.dma_start(
                        out[row : row + 128, 0 : H // 2], ot[:, 0 : H // 2]
                    )
                    nc.scalar.dma_start(
                        out[row : row + 128, H // 2 : H], ot[:, H // 2 : H]
                    )

            n = len(tiles)
            for i in range(n):
                stage1(*tiles[i])
                if i == D1:
                    # Bulk weights enter the sync ring here: behind
                    # xt(1..D1) (earlier deadlines), ahead of xt(D1+1...).
                    for k in range(KT):
                        nc.sync.dma_start(cw_sb[:, k], cw[:, k])
                    for oo in range(CT):
                        nc.sync.dma_start(wo_sb[:, oo], wo[:, oo])
                if i >= D1:
                    stage2(*tiles[i - D1])
                if i >= D1 + D2:
                    stage3(*tiles[i - D1 - D2])
            for i in range(n - D1, n):
                stage2(*tiles[i])
                stage3(*tiles[i - D2])
            for i in range(n - D2, n):
                stage3(*tiles[i])

    nc.compile()
    return nc


def _prep_inputs(x, W_in, b_in, conv_w, conv_b, W_out):
    """Host-side shard + transpose + cast. Returns in_maps for 8 cores."""
    x = np.asarray(x, dtype=np.float32)
    xr = x.reshape(T, HK, 128).transpose(2, 1, 0)  # [128, HK, T]
    xr16 = np.ascontiguousarray(xr[:, :HK16, :] * SCALE_X).astype(_BF16)
    xr8 = np.ascontiguousarray(
        np.clip(xr[:, HK16:, :] * SCALE_X, -240, 240)
    ).astype(_F8)

    in_maps = []
    for g in range(NCORES):
        c0 = g * CG
        w_in_g = (
            np.asarray(W_in[c0 : c0 + CG, :])
            .reshape(CT, 128, HK, 128)
            .transpose(3, 0, 2, 1)
        ) * SCALE_W  # [128, CT, HK, 128]: (hi, cc, hk, ci)
        w16_g = np.ascontiguousarray(w_in_g[:, :, :HK16, :]).astype(_BF16)
        w8_g = np.ascontiguousarray(
            np.clip(w_in_g[:, :, HK16:, :], -240, 240)
        ).astype(_F8)
        cw_g = np.ascontiguousarray(
            np.asarray(conv_w[c0 : c0 + CG, :, :])
            .reshape(CG, CT, 128, KT)
            .transpose(2, 3, 1, 0)
            .astype(_BF16)
        )  # [128, KT, CT, CG]: (ii, k, io, o) = conv_w[c0+o, io*128+ii, k]
        wo_g = np.ascontiguousarray(
            np.asarray(W_out[:, c0 : c0 + CG])
            .reshape(H, CT, 128)
            .transpose(2, 1, 0)
            .astype(_BF16)
        )  # [128, CT, H]: (oi, oo, h) = W_out[h, c0+oo*128+oi]
        bin_g = np.ascontiguousarray(
            np.asarray(b_in[c0 : c0 + CG], dtype=np.float32).reshape(CT, 128).T
        )  # [128, CT]
        cb_g = np.ascontiguousarray(
            np.asarray(conv_b[c0 : c0 + CG], dtype=np.float32).reshape(CT, 128).T
        )
        in_maps.append(
            {
                "xT16": xr16,
                "xT8": xr8,
                "w_in16": w16_g,
                "w_in8": w8_g,
                "cw": cw_g,
                "wo": wo_g,
                "b_in": bin_g,
                "cb": cb_g,
            }
        )
    return in_maps


def kernel(x, W_in, b_in, conv_w, conv_b, W_out, b_out):
    global LAST_RESULTS
    from concourse import bass_utils

    if "nc" not in _CACHE:
        _CACHE["nc"] = _build_nc()
    nc = _CACHE["nc"]

    in_maps = _prep_inputs(x, W_in, b_in, conv_w, conv_b, W_out)

    res = bass_utils.run_bass_kernel_spmd(
        nc, in_maps, core_ids=list(range(NCORES))
    )
    LAST_RESULTS = res

    acc = np.asarray(res.results[0]["out"]).astype(np.float32)
    for r in res.results[1:]:
        acc += np.asarray(r["out"]).astype(np.float32)
    acc += np.asarray(b_out, dtype=np.float32)[None, :]
    return acc.reshape(B, S, H)


# revision 15
# speedup vs baseline: 1.0705x; 1.0010x over previous
"""Trainium2 Bass kernel for nn_CausalConvolution (dense_cnn).

Reference computation (B=4, S=4096, H=2048, CIN=COUT=4096, K=4, G=8):
    h   = x @ W_in.T + b_in                       # [B,S,CIN]
    y   = silu(causal_grouped_conv1d(h) + conv_b) # [B,S,COUT], groups=8, k=4
    out = y @ W_out.T + b_out                     # [B,S,H]

Sharding: one conv group per NeuronCore (G = 8 = n_cores).
Core g computes channels [g*512, (g+1)*512) of h (column-parallel W_in),
its conv group (512 in / 512 out channels), and a row-parallel partial of
the output projection. Host sums the 8 partials (stored bf16) and adds
b_out. No cross-core communication on device.

All matmuls run with the contraction dim on SBUF partitions in a
"transposed" [channel, time] layout, bf16 with fp32 PSUM accumulation —
except the last N8 (=4) of stage 1's 16 contraction chunks, which run as
fp8e4 DoubleRow pairs (2 chunks per matmul, 2x PE throughput). To let
fp8 and bf16 products share one PSUM accumulation, ALL stage-1 operands
are pre-scaled by powers of two on the host (x*8, W_in*256; exact in
bf16), and the stage-1 activation applies 1/2048. rel_err budget: e4m3
on 4/16 chunks costs ~1.7e-2 of the 2e-2 allowance (measured in sim).

Schedule notes (from perfetto analysis of prior revisions):
- PE is the bottleneck: 5632 bf16 N=512 matmuls + 256 fp8 DoubleRow
  matmuls/core ~= 1.26 ms streaming floor. Everything else hides behind
  it or dies trying.
- DMA descriptors drain roughly FIFO per hwdge ring with bandwidth
  shared across all in-flight descriptors, so ISSUE ORDER is the
  scheduling tool: the sync ring carries x tiles (+ half the output
  stores), the scalar ring carries all weights in deadline order
  (w_in c0, c1, bias, c2, c3, conv, out) + the other half of stores.
- The PE warmup scratch must come from the persistent weight pool: a
  scratch in its own pool gets its SBUF reused for w_in, and the WAR
  dependency then blocks the critical first weight DMA until warmup
  ends (cost ~10 us, found the hard way).
- Stage 1 runs 3 tiles ahead of stage 2, stage 3 one tile behind
  stage 2, so the conv/out weights' arrival deadlines (~30/~45 us)
  clear while the PE chews through x-only work at the start.
"""

import numpy as np
import ml_dtypes

# Problem constants (hardcoded per the harness contract).
B, S, H = 4, 4096, 2048
CIN = COUT = 4096
KT = 4          # conv taps
G = 8           # conv groups == number of cores
CG = CIN // G   # 512 channels per group/core
T = B * S       # 16384 flattened time steps
NCORES = 8

HK = H // 128       # 16 contraction chunks for stage 1
N8 = 4              # stage-1 chunks done in fp8 DoubleRow (must be even)
HK16 = HK - N8      # stage-1 chunks done in bf16
CT = CG // 128      # 4 chunks of the per-core channel dim
TTILE = 512         # time-tile (N of every matmul)
NH = H // TTILE     # 4 output-column chunks of stage 3

SCALE_X = 8.0       # stage-1 operand pre-scales (powers of two, exact in bf16)
SCALE_W = 256.0
SCALE_INV = 1.0 / (SCALE_X * SCALE_W)

D1 = 3              # stage2 runs D1 tiles behind stage1
D2 = 1              # stage3 runs D2 tiles behind stage2

_BF16 = ml_dtypes.bfloat16
_F8 = ml_dtypes.float8_e4m3  # IEEE e4m3, max +-240 == TRN FP8_EXP4

_CACHE = {}

# test.py introspection: the most recent BassKernelResults from a run.
LAST_RESULTS = None


def _build_nc():
    import concourse.bass as bass
    import concourse.mybir as mybir
    import concourse.tile as tile
    from concourse import bacc

    dt = mybir.dt
    AF = mybir.ActivationFunctionType
    DR = mybir.MatmulPerfMode.DoubleRow

    nc = bacc.Bacc(
        "TRN2", target_bir_lowering=False, debug=False, num_devices=NCORES
    )

    xT16 = nc.dram_tensor("xT16", [128, HK16, T], dt.bfloat16, kind="ExternalInput")
    xT8 = nc.dram_tensor("xT8", [128, N8, T], dt.float8e4, kind="ExternalInput")
    w_in16 = nc.dram_tensor(
        "w_in16", [128, CT, HK16, 128], dt.bfloat16, kind="ExternalInput"
    )
    w_in8 = nc.dram_tensor(
        "w_in8", [128, CT, N8, 128], dt.float8e4, kind="ExternalInput"
    )
    cw = nc.dram_tensor("cw", [128, KT, CT, CG], dt.bfloat16, kind="ExternalInput")
    wo = nc.dram_tensor("wo", [128, CT, H], dt.bfloat16, kind="ExternalInput")
    b_in = nc.dram_tensor("b_in", [128, CT], dt.float32, kind="ExternalInput")
    cb = nc.dram_tensor("cb", [128, CT], dt.float32, kind="ExternalInput")
    # Per-core partials are summed on the host in fp32; storing them in
    # bf16 halves the store traffic and costs ~2e-4 extra rel err.
    out = nc.dram_tensor("out", [T, H], dt.bfloat16, kind="ExternalOutput")

    n_tt = S // TTILE  # time tiles per batch

    with tile.TileContext(nc) as tc:
        # Few pools: the kernel-exit barrier/drain chain costs ~1 us per
        # pool, so stream tiles share one pool via per-tag buf counts.
        with (
            tc.tile_pool(name="weights", bufs=1) as wpool,
            tc.tile_pool(name="stream", bufs=1) as spool,
            tc.tile_pool(name="psum", bufs=1, space="PSUM") as pspool,
        ):
            xpool = x8pool = hpool = ypool = opool = spool
            ps1pool = ps2pool = ps3pool = pspool
            # PE warmup: dep-free matmuls on scratch run while the first
            # weight/x DMAs are in flight, so HAM un-throttles (K=8/8)
            # before the real matmul stream begins. The scratch lives in
            # the persistent pool — see module docstring.
            scratch = wpool.tile([128, 640], dt.bfloat16)
            nc.vector.memset(scratch[:], 0.0)
            wps = ps3pool.tile([128, TTILE], dt.float32, tag="ps3", bufs=4)
            for _ in range(10):
                nc.tensor.matmul(
                    wps[:], scratch[:, 0:128], scratch[:, 128:640],
                    start=True, stop=True,
                )

            # ---- startup DMAs: ring order == drain order == priority ----
            # The startup is HBM-bandwidth-bound (stage 1 runs D1 tiles
            # ahead, consuming x at ~4x the steady rate), so descriptors
            # are ordered by consumption deadline. Weights go on the sync
            # ring; the first x tile on the scalar ring, which must be
            # drained of DMA issues before the first ACTIVATE needs it
            # (DMA backpressure on the issuing queue blocks later queue
            # entries). cw/wo are emitted mid-loop (after stage1(3)) so
            # they ride behind xt(1..3) in the sync ring.
            xt_first = xpool.tile([128, HK16, TTILE], dt.bfloat16, tag="xt", bufs=4)
            x8_first = x8pool.tile([128, N8, TTILE], dt.float8e4, tag="xt8", bufs=4)
            for q in range(4):
                nc.scalar.dma_start(
                    xt_first[:, 3 * q : 3 * q + 3, :],
                    xT16[:, 3 * q : 3 * q + 3, 0:TTILE],
                )
            nc.scalar.dma_start(x8_first[:], xT8[:, :, 0:TTILE])
            w16_sb = wpool.tile([128, CT, HK16, 128], dt.bfloat16)
            w8_sb = wpool.tile([128, CT, N8, 128], dt.float8e4)
            bin_sb = wpool.tile([128, CT], dt.float32)
            cb_sb = wpool.tile([128, CT], dt.float32)
            cw_sb = wpool.tile([128, KT, CT, CG], dt.bfloat16)
            wo_sb = wpool.tile([128, CT, H], dt.bfloat16)
            for half in range(2):
                nc.sync.dma_start(
                    w16_sb[:, 0, 6 * half : 6 * half + 6, :],
                    w_in16[:, 0, 6 * half : 6 * half + 6, :],
                )
            nc.sync.dma_start(w8_sb[:], w_in8[:])
            nc.sync.dma_start(w16_sb[:, 1], w_in16[:, 1])
            nc.sync.dma_start(bin_sb[:], b_in[:])
            nc.sync.dma_start(cb_sb[:], cb[:])
            nc.sync.dma_start(w16_sb[:, 2], w_in16[:, 2])
            nc.sync.dma_start(w16_sb[:, 3], w_in16[:, 3])

            tiles = [(b, tt) for b in range(B) for tt in range(n_tt)]
            hts = {}   # batch -> hT tile
            yts = {}   # (b, tt) -> y tile

            def stage1(b, tt):
                t0 = tt * TTILE
                tg = b * S + t0
                if tt == 0:
                    # h^T for this batch: [c, t] with a 3-column zero halo
                    # in front so causal taps at batch start read zeros.
                    hts[b] = hpool.tile(
                        [128, CT, KT - 1 + S], dt.bfloat16, tag="hT", name="hT",
                        bufs=2,
                    )
                    nc.vector.memset(hts[b][:, :, 0 : KT - 1], 0.0)
                hT = hts[b]
                if b == 0 and tt == 0:
                    xt, x8t = xt_first, x8_first
                else:
                    xt = xpool.tile([128, HK16, TTILE], dt.bfloat16, tag="xt", bufs=4)
                    x8t = x8pool.tile([128, N8, TTILE], dt.float8e4, tag="xt8", bufs=4)
                    nc.sync.dma_start(xt[:, 0:6, :], xT16[:, 0:6, tg : tg + TTILE])
                    nc.sync.dma_start(xt[:, 6:12, :], xT16[:, 6:12, tg : tg + TTILE])
                    nc.sync.dma_start(x8t[:], xT8[:, :, tg : tg + TTILE])
                for c in range(CT):
                    ps = ps1pool.tile([128, TTILE], dt.float32, tag="ps1", bufs=2)
                    for hk in range(HK16):
                        nc.tensor.matmul(
                            ps[:],
                            w16_sb[:, c, hk, :],
                            xt[:, hk, :],
                            start=(hk == 0),
                            stop=False,
                        )
                    for j in range(N8 // 2):
                        nc.tensor.matmul(
                            ps[:],
                            w8_sb[:, c, 2 * j : 2 * j + 2, :],
                            x8t[:, 2 * j : 2 * j + 2, :],
                            start=False,
                            stop=(j == N8 // 2 - 1),
                            perf_mode=DR,
                        )
                    nc.scalar.activation(
                        hT[:, c, KT - 1 + t0 : KT - 1 + t0 + TTILE],
                        ps[:],
                        AF.Identity,
                        bias=bin_sb[:, c : c + 1],
                        scale=SCALE_INV,
                    )

            def stage2(b, tt):
                t0 = tt * TTILE
                hT = hts[b]
                # causal grouped conv as 16 accumulated matmuls per chunk
                yt = ypool.tile([128, CT, TTILE], dt.bfloat16, tag="yt", bufs=3)
                yts[(b, tt)] = yt
                for o in range(CT):
                    ps = ps2pool.tile([128, TTILE], dt.float32, tag="ps2", bufs=2)
                    n_acc = KT * CT
                    acc = 0
                    for ik in range(CT):
                        for k in range(KT):
                            nc.tensor.matmul(
                                ps[:],
                                cw_sb[:, k, ik, o * 128 : (o + 1) * 128],
                                hT[:, ik, t0 + k : t0 + k + TTILE],
                                start=(acc == 0),
                                stop=(acc == n_acc - 1),
                            )
                            acc += 1
                    nc.scalar.activation(
                        yt[:, o, :],
                        ps[:],
                        AF.Silu,
                        bias=cb_sb[:, o : o + 1],
                    )

            def stage3(b, tt):
                tg = b * S + tt * TTILE
                yt = yts.pop((b, tt))
                for ss in range(TTILE // 128):
                    ot = opool.tile([128, H], dt.bfloat16, tag="ot", bufs=4)
                    for nh in range(NH):
                        ps = ps3pool.tile([128, TTILE], dt.float32, tag="ps3", bufs=4)
                        for oo in range(CT):
                            nc.tensor.matmul(
                                ps[:],
                                yt[:, oo, ss * 128 : (ss + 1) * 128],
                                wo_sb[:, oo, nh * TTILE : (nh + 1) * TTILE],
                                start=(oo == 0),
                                stop=(oo == CT - 1),
                            )
                        nc.vector.tensor_copy(
                            ot[:, nh * TTILE : (nh + 1) * TTILE], ps[:]
                        )
                    row = tg + ss * 128
                    # Two half-row stores, one per hwdge ring.
                    nc.sync.dma_start(
                        out[row : row + 128, 0 : H // 2], ot[:, 0 : H // 2]
                    )
                    nc.scalar.dma_start(
                        out[row : row + 128, H // 2 : H], ot[:, H // 2 : H]
                    )

            n = len(tiles)
            for i in range(n):
                stage1(*tiles[i])
                if i == D1:
                    # Bulk weights enter the sync ring here: behind
                    # xt(1..D1) (earlier deadlines), ahead of xt(D1+1...).
                    for k in range(KT):
                        nc.sync.dma_start(cw_sb[:, k], cw[:, k])
                    for oo in range(CT):
                        nc.sync.dma_start(wo_sb[:, oo], wo[:, oo])
                if i >= D1:
                    stage2(*tiles[i - D1])
                if i >= D1 + D2:
                    stage3(*tiles[i - D1 - D2])
            for i in range(n - D1, n):
                stage2(*tiles[i])
                stage3(*tiles[i - D2])
            for i in range(n - D2, n):
                stage3(*tiles[i])

    nc.compile()
    return nc


def _prep_inputs(x, W_in, b_in, conv_w, conv_b, W_out):
    """Host-side shard + transpose + cast. Returns in_maps for 8 cores."""
    x = np.asarray(x, dtype=np.float32)
    xr = x.reshape(T, HK, 128).transpose(2, 1, 0)  # [128, HK, T]
    xr16 = np.ascontiguousarray(xr[:, :HK16, :] * SCALE_X).astype(_BF16)
    xr8 = np.ascontiguousarray(
        np.clip(xr[:, HK16:, :] * SCALE_X, -240, 240)
    ).astype(_F8)

    in_maps = []
    for g in range(NCORES):
        c0 = g * CG
        w_in_g = (
            np.asarray(W_in[c0 : c0 + CG, :])
            .reshape(CT, 128, HK, 128)
            .transpose(3, 0, 2, 1)
        ) * SCALE_W  # [128, CT, HK, 128]: (hi, cc, hk, ci)
        w16_g = np.ascontiguousarray(w_in_g[:, :, :HK16, :]).astype(_BF16)
        w8_g = np.ascontiguousarray(
            np.clip(w_in_g[:, :, HK16:, :], -240, 240)
        ).astype(_F8)
        cw_g = np.ascontiguousarray(
            np.asarray(conv_w[c0 : c0 + CG, :, :])
            .reshape(CG, CT, 128, KT)
            .transpose(2, 3, 1, 0)
            .astype(_BF16)
        )  # [128, KT, CT, CG]: (ii, k, io, o) = conv_w[c0+o, io*128+ii, k]
        wo_g = np.ascontiguousarray(
            np.asarray(W_out[:, c0 : c0 + CG])
            .reshape(H, CT, 128)
            .transpose(2, 1, 0)
            .astype(_BF16)
        )  # [128, CT, H]: (oi, oo, h) = W_out[h, c0+oo*128+oi]
        bin_g = np.ascontiguousarray(
            np.asarray(b_in[c0 : c0 + CG], dtype=np.float32).reshape(CT, 128).T
        )  # [128, CT]
        cb_g = np.ascontiguousarray(
            np.asarray(conv_b[c0 : c0 + CG], dtype=np.float32).reshape(CT, 128).T
        )
        in_maps.append(
            {
                "xT16": xr16,
                "xT8": xr8,
                "w_in16": w16_g,
                "w_in8": w8_g,
                "cw": cw_g,
                "wo": wo_g,
                "b_in": bin_g,
                "cb": cb_g,
            }
        )
    return in_maps


def kernel(x, W_in, b_in, conv_w, conv_b, W_out, b_out):
    global LAST_RESULTS
    from concourse import bass_utils

    if "nc" not in _CACHE:
        _CACHE["nc"] = _build_nc()
    nc = _CACHE["nc"]

    in_maps = _prep_inputs(x, W_in, b_in, conv_w, conv_b, W_out)

    res = bass_utils.run_bass_kernel_spmd(
        nc, in_maps, core_ids=list(range(NCORES))
    )
    LAST_RESULTS = res

    acc = np.asarray(res.results[0]["out"]).astype(np.float32)
    for r in res.results[1:]:
        acc += np.asarray(r["out"]).astype(np.float32)
    acc += np.asarray(b_out, dtype=np.float32)[None, :]
    return acc.reshape(B, S, H)
